# revision 11
# baseline (speedup 1.0000x reference)
"""Trainium2 kernel for nn_DilatedReparamBlock_21139829031531.

Math: the block is 7 depthwise-conv+BN branches summed. All branches merge
("reparameterize") exactly into ONE depthwise 13x13 conv + per-channel bias:
    K_c = sum_i scale_i[c] * dilate(w_i[c])   (placed centered in 13x13)
    bias_c = sum_i (beta_i - mean_i * scale_i)[c]
The host folds weights/BN (tiny), the device runs the single conv.

Device scheme ("row-pair Toeplitz"): channels sharded 32/core across 8 cores.
Per channel, SBUF holds x with partitions = (h%2)*56 + w (a row-PAIR of the
image across 112 partitions; partition 112 = constant 1.0 for the bias).
The 13x13 depthwise conv becomes 7 PE matmuls accumulating in PSUM:
    out[(a_out,w_out), (b,t)] += sum_{a_in,w_in}
        lhsT_j[(a_in,w_in), (a_out,w_out)] * x[b, 2(t+j2)+a_in, w_in]
where lhsT_j[(ai,wi),(ao,wo)] = K[2*j2+ai-ao+6, wi-wo+6] (0 outside the
13-tap band; W/H zero-padding falls out of band clipping + t clipping).
fp16 operands, fp32 PSUM accumulate, fp32 output.
"""

import os
import sys

import numpy as np

for _p in ("/opt/trn_rl_repo", "/root/.axon_site/_ro/trn_rl_repo"):
    if os.path.isdir(_p) and _p not in sys.path:
        sys.path.append(_p)

import concourse.bass as bass
import concourse.bacc as bacc
import concourse.mybir as mybir
from concourse import tile
from concourse.vector_clock import ScopedClock
from concourse.bass_utils import run_bass_kernel_spmd
import bass_rust as _br

# ---------------------------------------------------------------- constants
C = 256
B = 32
H = W = 56
M = H // 2            # row pairs
NCORES = 8
CL = C // NCORES      # channels per core
KS = [5, 7, 7, 3, 3, 3]
DIL = [1, 1, 2, 3, 4, 5]
EPS = 1e-5
J2S = [0, -3, -2, -1, 1, 2, 3]   # j2 = (row-pair offset)/1; row offset = 2*j2
GROUPS = 16
GC = CL // GROUPS     # channels per group
F16 = mybir.dt.float16
F32 = mybir.dt.float32

# ------------------------------------------------- tile drain-limit patch
# This walrus build allows at most ONE sem wait per instruction; the stock
# TileContext tail drain stacks all pending waits onto a single Drain and
# fails codegen ("Too many sync wait commands"). Split the extras onto
# standalone sequencer waits.
def _patched_drain_and_barrier(self, tick_clock, wait_clock):
    nc = self.nc
    drain_inst = nc.sync.drain()
    wait_clock.add_sem_waits(
        drain_inst.ins, ScopedClock({None: tick_clock.global_clock})
    )
    si = drain_inst.ins.sync_info
    if si is not None and len(si.on_wait) > 1:
        waits = list(si.on_wait)
        drain_inst.ins.sync_info = _br.SyncInfo(
            on_wait=[waits[0]], on_update=list(si.on_update)
        )
        by_num = {h.num: h for h in self.sems.allocated().values()}
        for w in waits[1:]:
            nc.sync.wait_ge(by_num[w.id], w.wait_value)
    nc.all_engine_barrier()
    assert self.sems is not None
    popped = nc._tile_sem_poison_stack.pop()
    assert popped is self._sem_poison
    nc.clear_and_free_semaphores(list(self.sems.allocated().values()))
    nc.all_engine_barrier()


if getattr(tile.TileContext._drain_and_barrier, "__name__", "") != (
    "_patched_drain_and_barrier"
):
    tile.TileContext._drain_and_barrier = _patched_drain_and_barrier


# ------------------------------------------------------------- host math
def _merge_weights(lk_w, ws, bn_gamma, bn_beta, bn_mean, bn_var):
    """Fold all branches + BN into one [C,13,13] kernel and [C] bias."""
    g = bn_gamma.astype(np.float64)
    be = bn_beta.astype(np.float64)
    mu = bn_mean.astype(np.float64)
    va = bn_var.astype(np.float64)
    scale = g / np.sqrt(va + EPS)          # [7, C]
    shift = be - mu * scale                # [7, C]
    K = np.zeros((C, 13, 13), np.float64)
    K += scale[0][:, None, None] * lk_w[:, 0].astype(np.float64)
    for i, (k, r) in enumerate(zip(KS, DIL)):
        w = ws[i][:, 0].astype(np.float64)
        span = r * (k - 1) + 1
        off = (13 - span) // 2
        ii = off + r * np.arange(k)
        K[:, ii[:, None], ii[None, :]] += scale[i + 1][:, None, None] * w
    bias = shift.sum(axis=0)               # [C]
    return K, bias


def _build_toeplitz(K, bias):
    """lhsT for all channels: [C, 7, 112, 112] fp16.

    lhsT[c, j, p=(ai*56+wi), f=(ao*56+wo)] = K[c, 2*j2+ai-ao+6, wi-wo+6]
    """
    p = np.arange(112)
    f = np.arange(112)
    ai, wi = p // 56, p % 56
    ao, wo = f // 56, f % 56
    dx = wi[:, None] - wo[None, :]                       # [112,112]
    lhs = np.zeros((C, 7, 112, 112), np.float32)
    for j, j2 in enumerate(J2S):
        dy = 2 * j2 + ai[:, None] - ao[None, :]          # [112,112]
        valid = (np.abs(dy) <= 6) & (np.abs(dx) <= 6)
        dyc = np.clip(dy + 6, 0, 12)
        dxc = np.clip(dx + 6, 0, 12)
        vals = K[:, dyc, dxc] * valid[None]              # [C,112,112]
        lhs[:, j] = vals
    return lhs.astype(np.float16)


def _stage_inputs(x, K, bias):
    """Per-core xs [112, CL, B, M] fp16, wt [112, CL, 7, 128] fp16, and a
    bias plane [112, CL] fp32 (bias replicated across partitions so the
    PSUM->SBUF copy can add it as a per-partition tensor_scalar operand).
    Partition counts stay multiples of 16 — the DMA engine striping
    serializes onto one SDMA engine otherwise."""
    lhs = _build_toeplitz(K, bias)                       # [C, 7, 112, 128]
    xr = (
        x.reshape(B, NCORES, CL, M, 2, W)
        .transpose(4, 5, 1, 2, 0, 3)                     # [a, w, core, cl, b, m]
        .reshape(112, NCORES, CL, B, M)
        .astype(np.float16)
    )
    xs_l, wt_l, bi_l = [], [], []
    for core in range(NCORES):
        xs_l.append(np.ascontiguousarray(xr[:, core]))
        wc = lhs[core * CL:(core + 1) * CL].transpose(2, 0, 1, 3)  # [112,CL,7,112]
        wt_l.append(np.ascontiguousarray(wc))
        bc = np.broadcast_to(
            bias[core * CL:(core + 1) * CL].astype(np.float32)[None, :], (112, CL)
        )
        bi_l.append(np.ascontiguousarray(bc))
    return xs_l, wt_l, bi_l


def _unstage(outs):
    """outs: list of 8 arrays [112, CL, B, M] fp16 -> [B, C, H, W] fp32."""
    O = np.stack(outs).astype(np.float32)                # [8, 112, CL, B, M]
    return np.ascontiguousarray(
        O.reshape(NCORES, 2, W, CL, B, M)
        .transpose(4, 0, 3, 5, 1, 2)                     # [B, core, cl, m, a, w]
        .reshape(B, C, H, W)
    )


# --------------------------------------------------------- device program
def _build_program():
    # Bacc (not plain Bass): its finalize() runs generate_event_semaphores,
    # which legalizes multi-sem waits into EVSEM chains (walrus allows only
    # one wait per instruction).
    nc = bacc.Bacc()
    xs = nc.declare_dram_parameter("xs", [112, CL, B, M], F16, isOutput=False)
    wt = nc.declare_dram_parameter("wt", [112, CL, 7, 112], F16, isOutput=False)
    bi = nc.declare_dram_parameter("bi", [112, CL], F32, isOutput=False)
    out = nc.declare_dram_parameter("out", [112, CL, B, M], F16, isOutput=True)

    with tile.TileContext(nc) as tc:
        with (
            tc.tile_pool(name="xp", bufs=4) as xp,
            tc.tile_pool(name="wp", bufs=4) as wp,
            tc.tile_pool(name="pp", bufs=4, space="PSUM") as pp,
            tc.tile_pool(name="op", bufs=4) as op,
            tc.tile_pool(name="bp", bufs=1) as bp,
        ):
            bias_t = bp.tile([112, CL], F32)
            nc.sync.dma_start(bias_t[:], bi[:])
            for g in range(GROUPS):
                c0 = g * GC
                xt = xp.tile([112, GC, B, M], F16)
                wtt = wp.tile([112, GC, 7, 112], F16)
                nc.sync.dma_start(xt[:], xs[:, c0:c0 + GC])
                nc.sync.dma_start(wtt[:], wt[:, c0:c0 + GC])
                for ci in range(GC):
                    pa = pp.tile([112, 16, M], F32)
                    pb = pp.tile([112, 16, M], F32)
                    for j, j2 in enumerate(J2S):
                        t0 = max(0, -j2)
                        t1 = M - max(0, j2)
                        for ps, b0 in ((pa, 0), (pb, 16)):
                            nc.tensor.matmul(
                                ps[:, :, t0:t1],
                                wtt[:, ci, j, :],
                                xt[:, ci, b0:b0 + 16, t0 + j2:t1 + j2],
                                start=(j == 0),
                                stop=(j == len(J2S) - 1),
                            )
                    for ps, b0 in ((pa, 0), (pb, 16)):
                        sb = op.tile([112, 16, M], F16)
                        nc.vector.tensor_scalar_add(
                            sb[:], ps[:], bias_t[:, c0 + ci:c0 + ci + 1]
                        )
                        nc.sync.dma_start(
                            out[:, c0 + ci, b0:b0 + 16, :], sb[:]
                        )
    nc.finalize()
    return nc


_NC_CACHE = None
LAST_RESULTS = None   # test harness introspection


def kernel(x, lk_w, w0, w1, w2, w3, w4, w5, bn_gamma, bn_beta, bn_mean,
           bn_var):
    global _NC_CACHE, LAST_RESULTS
    x = np.asarray(x, np.float32)
    K, bias = _merge_weights(
        np.asarray(lk_w), [np.asarray(w) for w in (w0, w1, w2, w3, w4, w5)],
        np.asarray(bn_gamma), np.asarray(bn_beta), np.asarray(bn_mean),
        np.asarray(bn_var))
    xs_l, wt_l, bi_l = _stage_inputs(x, K, bias)
    if _NC_CACHE is None:
        _NC_CACHE = _build_program()
    nc = _NC_CACHE
    in_maps = [
        {"xs": xs_l[i], "wt": wt_l[i], "bi": bi_l[i]} for i in range(NCORES)
    ]
    trace = bool(int(os.environ.get("DRB_TRACE", "0")))
    res = run_bass_kernel_spmd(nc, in_maps, list(range(NCORES)), trace=trace)
    LAST_RESULTS = res
    return _unstage([res.results[i]["out"] for i in range(NCORES)])


# revision 13
# speedup vs baseline: 1.0328x; 1.0328x over previous
"""Trainium2 kernel for nn_DilatedReparamBlock_21139829031531.

Math: the block is 7 depthwise-conv+BN branches summed. All branches merge
("reparameterize") exactly into ONE depthwise 13x13 conv + per-channel bias:
    K_c = sum_i scale_i[c] * dilate(w_i[c])   (placed centered in 13x13)
    bias_c = sum_i (beta_i - mean_i * scale_i)[c]
The host folds weights/BN (tiny), the device runs the single conv.

Device scheme ("row-pair Toeplitz"): channels sharded 32/core across 8 cores.
Per channel, SBUF holds x with partitions = (h%2)*56 + w (a row-PAIR of the
image across 112 partitions; partition 112 = constant 1.0 for the bias).
The 13x13 depthwise conv becomes 7 PE matmuls accumulating in PSUM:
    out[(a_out,w_out), (b,t)] += sum_{a_in,w_in}
        lhsT_j[(a_in,w_in), (a_out,w_out)] * x[b, 2(t+j2)+a_in, w_in]
where lhsT_j[(ai,wi),(ao,wo)] = K[2*j2+ai-ao+6, wi-wo+6] (0 outside the
13-tap band; W/H zero-padding falls out of band clipping + t clipping).
fp16 operands, fp32 PSUM accumulate, fp32 output.
"""

import os
import sys

import numpy as np

for _p in ("/opt/trn_rl_repo", "/root/.axon_site/_ro/trn_rl_repo"):
    if os.path.isdir(_p) and _p not in sys.path:
        sys.path.append(_p)

import concourse.bass as bass
import concourse.bacc as bacc
import concourse.mybir as mybir
from concourse import tile
from concourse.vector_clock import ScopedClock
from concourse.bass_utils import run_bass_kernel_spmd
import bass_rust as _br

# ---------------------------------------------------------------- constants
C = 256
B = 32
H = W = 56
M = H // 2            # row pairs
NCORES = 8
CL = C // NCORES      # channels per core
KS = [5, 7, 7, 3, 3, 3]
DIL = [1, 1, 2, 3, 4, 5]
EPS = 1e-5
J2S = [0, -3, -2, -1, 1, 2, 3]   # j2 = (row-pair offset)/1; row offset = 2*j2
GROUPS = 16
GC = CL // GROUPS     # channels per group
F16 = mybir.dt.float16
F32 = mybir.dt.float32

# ------------------------------------------------- tile drain-limit patch
# This walrus build allows at most ONE sem wait per instruction; the stock
# TileContext tail drain stacks all pending waits onto a single Drain and
# fails codegen ("Too many sync wait commands"). Split the extras onto
# standalone sequencer waits.
def _patched_drain_and_barrier(self, tick_clock, wait_clock):
    nc = self.nc
    drain_inst = nc.sync.drain()
    wait_clock.add_sem_waits(
        drain_inst.ins, ScopedClock({None: tick_clock.global_clock})
    )
    si = drain_inst.ins.sync_info
    if si is not None and len(si.on_wait) > 1:
        waits = list(si.on_wait)
        drain_inst.ins.sync_info = _br.SyncInfo(
            on_wait=[waits[0]], on_update=list(si.on_update)
        )
        by_num = {h.num: h for h in self.sems.allocated().values()}
        for w in waits[1:]:
            nc.sync.wait_ge(by_num[w.id], w.wait_value)
    nc.all_engine_barrier()
    assert self.sems is not None
    popped = nc._tile_sem_poison_stack.pop()
    assert popped is self._sem_poison
    nc.clear_and_free_semaphores(list(self.sems.allocated().values()))
    nc.all_engine_barrier()


if getattr(tile.TileContext._drain_and_barrier, "__name__", "") != (
    "_patched_drain_and_barrier"
):
    tile.TileContext._drain_and_barrier = _patched_drain_and_barrier


# ------------------------------------------------------------- host math
def _merge_weights(lk_w, ws, bn_gamma, bn_beta, bn_mean, bn_var):
    """Fold all branches + BN into one [C,13,13] kernel and [C] bias."""
    g = bn_gamma.astype(np.float64)
    be = bn_beta.astype(np.float64)
    mu = bn_mean.astype(np.float64)
    va = bn_var.astype(np.float64)
    scale = g / np.sqrt(va + EPS)          # [7, C]
    shift = be - mu * scale                # [7, C]
    K = np.zeros((C, 13, 13), np.float64)
    K += scale[0][:, None, None] * lk_w[:, 0].astype(np.float64)
    for i, (k, r) in enumerate(zip(KS, DIL)):
        w = ws[i][:, 0].astype(np.float64)
        span = r * (k - 1) + 1
        off = (13 - span) // 2
        ii = off + r * np.arange(k)
        K[:, ii[:, None], ii[None, :]] += scale[i + 1][:, None, None] * w
    bias = shift.sum(axis=0)               # [C]
    return K, bias


def _build_toeplitz(K, bias):
    """lhsT for all channels: [C, 7, 112, 112] fp16.

    lhsT[c, j, p=(ai*56+wi), f=(ao*56+wo)] = K[c, 2*j2+ai-ao+6, wi-wo+6]
    """
    p = np.arange(112)
    f = np.arange(112)
    ai, wi = p // 56, p % 56
    ao, wo = f // 56, f % 56
    dx = wi[:, None] - wo[None, :]                       # [112,112]
    lhs = np.zeros((C, 7, 112, 112), np.float32)
    for j, j2 in enumerate(J2S):
        dy = 2 * j2 + ai[:, None] - ao[None, :]          # [112,112]
        valid = (np.abs(dy) <= 6) & (np.abs(dx) <= 6)
        dyc = np.clip(dy + 6, 0, 12)
        dxc = np.clip(dx + 6, 0, 12)
        vals = K[:, dyc, dxc] * valid[None]              # [C,112,112]
        lhs[:, j] = vals
    return lhs.astype(np.float16)


def _stage_inputs(x, K, bias):
    """Per-core xs [112, CL, B, M] fp16, wt [112, CL, 7, 128] fp16, and a
    bias plane [112, CL] fp32 (bias replicated across partitions so the
    PSUM->SBUF copy can add it as a per-partition tensor_scalar operand).
    Partition counts stay multiples of 16 — the DMA engine striping
    serializes onto one SDMA engine otherwise."""
    lhs = _build_toeplitz(K, bias)                       # [C, 7, 112, 128]
    xr = (
        x.reshape(B, NCORES, CL, M, 2, W)
        .transpose(4, 5, 1, 2, 0, 3)                     # [a, w, core, cl, b, m]
        .reshape(112, NCORES, CL, B, M)
        .astype(np.float16)
    )
    xs_l, wt_l, bi_l = [], [], []
    for core in range(NCORES):
        xs_l.append(np.ascontiguousarray(xr[:, core]))
        wc = lhs[core * CL:(core + 1) * CL].transpose(2, 0, 1, 3)  # [112,CL,7,112]
        wt_l.append(np.ascontiguousarray(wc))
        bc = np.broadcast_to(
            bias[core * CL:(core + 1) * CL].astype(np.float32)[None, :], (112, CL)
        )
        bi_l.append(np.ascontiguousarray(bc))
    return xs_l, wt_l, bi_l


def _unstage(outs):
    """outs: list of 8 arrays [112, CL, B, M] fp16 -> [B, C, H, W] fp32."""
    O = np.stack(outs).astype(np.float32)                # [8, 112, CL, B, M]
    return np.ascontiguousarray(
        O.reshape(NCORES, 2, W, CL, B, M)
        .transpose(4, 0, 3, 5, 1, 2)                     # [B, core, cl, m, a, w]
        .reshape(B, C, H, W)
    )


# --------------------------------------------------------- device program
def _build_program():
    # Bacc (not plain Bass): its finalize() runs generate_event_semaphores,
    # which legalizes multi-sem waits into EVSEM chains (walrus allows only
    # one wait per instruction).
    nc = bacc.Bacc()
    xs = nc.declare_dram_parameter("xs", [112, CL, B, M], F16, isOutput=False)
    wt = nc.declare_dram_parameter("wt", [112, CL, 7, 112], F16, isOutput=False)
    bi = nc.declare_dram_parameter("bi", [112, CL], F32, isOutput=False)
    out = nc.declare_dram_parameter("out", [112, CL, B, M], F16, isOutput=True)

    with tile.TileContext(nc) as tc:
        with (
            tc.tile_pool(name="xp", bufs=4) as xp,
            tc.tile_pool(name="wp", bufs=4) as wp,
            tc.tile_pool(name="pp", bufs=4, space="PSUM") as pp,
            tc.tile_pool(name="op", bufs=4) as op,
            tc.tile_pool(name="bp", bufs=1) as bp,
        ):
            # PE warm-up: ~3.5us of matmuls on garbage SBUF while the first
            # input DMAs are in flight, so HAM un-throttles (1.2->2.4 GHz)
            # before the real matmul stream begins. PSUM target shares the
            # "pa" slots (WAW-ordered before first real use); never read.
            warm = op.tile([112, 512], F16, tag="warm")
            wps = pp.tile([112, 16, M], F32, tag="pa")
            nc.gpsimd.memset(warm[:], 1.0)
            for _ in range(16):
                nc.tensor.matmul(
                    wps[:], warm[:, 0:112], warm[:, 0:448],
                    start=True, stop=True,
                )
            bias_t = bp.tile([112, CL], F32)
            nc.sync.dma_start(bias_t[:], bi[:])
            for g in range(GROUPS):
                c0 = g * GC
                xt = xp.tile([112, GC, B, M], F16)
                wtt = wp.tile([112, GC, 7, 112], F16)
                nc.sync.dma_start(xt[:], xs[:, c0:c0 + GC])
                nc.sync.dma_start(wtt[:], wt[:, c0:c0 + GC])
                for ci in range(GC):
                    pa = pp.tile([112, 16, M], F32)
                    pb = pp.tile([112, 16, M], F32)
                    for j, j2 in enumerate(J2S):
                        t0 = max(0, -j2)
                        t1 = M - max(0, j2)
                        for ps, b0 in ((pa, 0), (pb, 16)):
                            nc.tensor.matmul(
                                ps[:, :, t0:t1],
                                wtt[:, ci, j, :],
                                xt[:, ci, b0:b0 + 16, t0 + j2:t1 + j2],
                                start=(j == 0),
                                stop=(j == len(J2S) - 1),
                            )
                    for ps, b0 in ((pa, 0), (pb, 16)):
                        sb = op.tile([112, 16, M], F16)
                        nc.vector.tensor_scalar_add(
                            sb[:], ps[:], bias_t[:, c0 + ci:c0 + ci + 1]
                        )
                        nc.sync.dma_start(
                            out[:, c0 + ci, b0:b0 + 16, :], sb[:]
                        )
    nc.finalize()
    return nc


_NC_CACHE = None
LAST_RESULTS = None   # test harness introspection


def kernel(x, lk_w, w0, w1, w2, w3, w4, w5, bn_gamma, bn_beta, bn_mean,
           bn_var):
    global _NC_CACHE, LAST_RESULTS
    x = np.asarray(x, np.float32)
    K, bias = _merge_weights(
        np.asarray(lk_w), [np.asarray(w) for w in (w0, w1, w2, w3, w4, w5)],
        np.asarray(bn_gamma), np.asarray(bn_beta), np.asarray(bn_mean),
        np.asarray(bn_var))
    xs_l, wt_l, bi_l = _stage_inputs(x, K, bias)
    if _NC_CACHE is None:
        _NC_CACHE = _build_program()
    nc = _NC_CACHE
    in_maps = [
        {"xs": xs_l[i], "wt": wt_l[i], "bi": bi_l[i]} for i in range(NCORES)
    ]
    trace = bool(int(os.environ.get("DRB_TRACE", "0")))
    res = run_bass_kernel_spmd(nc, in_maps, list(range(NCORES)), trace=trace)
    LAST_RESULTS = res
    return _unstage([res.results[i]["out"] for i in range(NCORES)])
